# revision 22
# baseline (speedup 1.0000x reference)
"""Trainium2 Bass kernel for nn_CapsuleLayer (dynamic routing capsule layer).

Reference computation:
    u_hat = einsum('jidk,bik->bjid', W, inputs)        # [B,J,I,D]
    b = 0
    for r in 0..2:
        c = softmax_j(b)                               # [B,J,1,I]
        s = einsum('bjoi,bjid->bjod', c, u_hat)        # [B,J,1,D]
        out = squash(s)
        if r < 2: b += einsum('bjod,bjid->bjoi', out, u_hat)
    return out                                         # [B,J,D]

Strategy: shard I (=2048) across 8 cores (I_loc=256), keep full B=128 on
every core.  u_hat (168 MB) is NEVER materialized: both routing
contractions are expressed against W directly:

    s[b,j,d]  = sum_{i,k} (c[b,j,i] * x[b,i,k]) * W[j,i,d,k]   (PE, K=(k,i))
    a[b,j,i]  = sum_k x[b,i,k] * T[b,j,k,i],
    T[b,j,k,i] = sum_d out[b,j,d] * W[j,i,d,k]                 (PE, K=d)

All routing state (c, products, y) lives in the [i%128-partition] layout
so the softmax/y path needs NO transposes; the T matmuls emit [i, b]
tiles directly (stationary = W slices, moving = out^T).  Softmax
normalization is kept as self-normalized state: c <- (c_prev * exp(a)) / Z,
valid because softmax is scale-invariant per (i, b).

One small fp16 AllReduce per routing iteration combines the per-core s
partial sums; a dummy 32-byte AllReduce issued at kernel start absorbs
cross-core launch skew so the real collectives see aligned cores.

Matmul and DVE operands are fp16 (fp32 matmuls cost a 2-way hi/lo split;
2-byte packed DVE ops run in the 2x perf mode).  PE accumulation stays
fp32 in PSUM; squash runs fp32.
"""

import numpy as np
from contextlib import ExitStack

import concourse.bass as bass
import concourse.bacc as bacc
import concourse.tile as tile
from concourse import mybir
from concourse.bass_utils import run_bass_kernel_spmd
from concourse.masks import make_identity

F32 = mybir.dt.float32
F16 = mybir.dt.float16  # 10-bit mantissa, fast PE weight load, DVE 2x mode
AX = mybir.AxisListType
OP = mybir.AluOpType
ACTF = mybir.ActivationFunctionType

B = 128       # batch
I = 2048      # input capsules (sharded)
K = 8         # DIN
J = 10        # output capsules
D = 16        # DOUT
R = 3         # routing iterations
NCORES = 8

K_EPS = 1e-7
NORM_EPS = 1e-6


def bcast(ap: bass.AP, n: int) -> bass.AP:
    """Append a stride-0 (broadcast) innermost free dim of size n."""
    return bass.AP(ap.tensor, ap.offset, [*ap.ap, [0, n]])


def _pin_activation_tables():
    """Make every activation function we use resolve to the one table set
    that contains them all (natural_log_exp_and_others), so the compiler
    emits a single ACT_TABLE_LOAD instead of thrashing between sets."""
    import concourse.hw_specs as hw_specs

    if getattr(bacc, "_capsule_tables_pinned", False):
        return
    orig = hw_specs.get_activation_tables
    mine = {"Exp", "Ln", "Copy", "Identity", "Square"}

    def patched(module_arch):
        tables = dict(orig(module_arch))
        out = {}
        for name, funcs in tables.items():
            if name == "natural_log_exp_and_others":
                out[name] = funcs
            else:
                out[name] = {f for f in funcs if f.name not in mine}
        return out

    bacc.get_activation_tables = patched
    bacc._capsule_tables_pinned = True


def build_nc(n_cores: int = NCORES):
    IL = I // NCORES          # 256 per-core input capsules (also for n_cores=1 sim)
    IT = IL // 128            # 2 partition tiles of i
    NCH = IL * K // 128       # 16 (k,i)-chunks of 128 contraction rows
    KH = K // 2               # T computed in two 4-k halves (PSUM budget)

    _pin_activation_tables()
    nc = bacc.Bacc(num_devices=n_cores)

    x_ext = nc.dram_tensor("x", [B, IL, K], F32, kind="ExternalInput")
    w_ext = nc.dram_tensor("w", [J, IL, D, K], F32, kind="ExternalInput")
    out_ext = nc.dram_tensor("out", [B, J, D], F32, kind="ExternalOutput")
    # fp16 all-reduce payloads, one pair per iteration
    ar_in = [nc.dram_tensor(f"ar_in{r}", [B, J * D], F16) for r in range(R)]
    ar_out = [
        nc.dram_tensor(f"ar_out{r}", [B, J * D], F16, addr_space="Shared")
        for r in range(R)
    ]

    with tile.TileContext(nc) as tc, ExitStack() as ctx:
        sb = ctx.enter_context(tc.tile_pool(name="sb", bufs=1))
        ypool = ctx.enter_context(tc.tile_pool(name="ypool", bufs=3))
        tpool = ctx.enter_context(tc.tile_pool(name="tpool", bufs=3))
        rpool = ctx.enter_context(tc.tile_pool(name="rpool", bufs=3))
        pst = ctx.enter_context(tc.tile_pool(name="pst", bufs=2, space="PSUM"))
        ps_s_pool = ctx.enter_context(tc.tile_pool(name="ps_s", bufs=2, space="PSUM"))
        ps_t_pool = ctx.enter_context(tc.tile_pool(name="ps_t", bufs=2, space="PSUM"))

        # (no dummy collective: the framework's prelude kernel barrier
        # already absorbs cross-core launch skew on the CC engine)
        ident = sb.tile([128, 128], F32)
        make_identity(nc, ident)
        ident_h = sb.tile([128, 128], F16)
        nc.vector.tensor_copy(out=ident_h[:], in_=ident[:])

        # persistent tensors
        x_kc = sb.tile([128, K, IL], F32)            # x[b, k, i] fp32 staging
        x_kc_h = sb.tile([128, K, IL], F16)          # fp16 copy (transpose src)
        x_t = sb.tile([128, K, IT, 128], F16)        # x^T: [(i%128), k, it, b]
        x_p = sb.tile([128, K, IT, 128], F16)        # x' = x / Z (per iteration)
        w_nat = sb.tile([128, IT, J, D * K], F32)    # w[i%128, it, j, (d,k)]
        w_bf = sb.tile([128, IT, J, D * K], F16)     # fp16 copy (s-matmul lhsT)
        w_shuf = sb.tile([128, IT, J, K * D], F16)   # (d,k) -> (k,d) shuffle fp16
        w_kd_k = sb.tile([16, K, J, IT, 128], F16)   # w[d, k, j, it, i] (base-0 rows)
        w_kd_bf = sb.tile([128, J, IT, 128], F16)    # staging for the re-base

        # routing state ([i%128] layout)
        c_st = sb.tile([128, IT, J, 128], F16)       # c[i%128, it, j, b]
        zt1 = sb.tile([128, IT, 5, 128], F16)
        zt2 = sb.tile([128, IT, 2, 128], F16)
        z_t = sb.tile([128, IT, 128], F16)
        rz_t = sb.tile([128, IT, 128], F16)

        # [b] layout state
        s_sb = sb.tile([128, J, D], F16)             # all-reduced s
        s_stage = sb.tile([128, J, D], F16)          # pre-allreduce staging (iter 0)
        sT_stage = sb.tile([16, J, 128], F16)        # pre-allreduce staging (iters 1+)
        sT_sb = sb.tile([16, J, 128], F16)           # all-reduced sT
        out_sb = sb.tile([128, J, D], F32)           # squash output
        out_hb = sb.tile([128, J, D], F16)           # fp16 copy (transpose src)
        outT = sb.tile([16, J, 128], F16)            # out^T: [d, j, b]

        # small squash temps
        sq_s2 = sb.tile([128, J], F32)
        sq_mean = sb.tile([128, J], F32)
        sq_t = sb.tile([128, J, D], F32)
        sq_sq = sb.tile([128, J, D], F32)
        sq_var = sb.tile([128, J], F32)
        sq_ln = sb.tile([128, J], F32)
        sq_rs = sb.tile([128, J], F32)
        sq_u = sb.tile([128, J], F32)
        sq_den = sb.tile([128, J], F32)
        sq_rden = sb.tile([128, J], F32)
        sq_scale = sb.tile([128, J], F32)
        sq_m2 = sb.tile([128, J], F32)
        eps_k = sb.tile([128, 1], F32)
        nc.vector.memset(eps_k[:], K_EPS)
        eps_n = sb.tile([128, 1], F32)
        nc.vector.memset(eps_n[:], NORM_EPS)

        # ---------------- load + relayout ----------------
        eng = [nc.sync, nc.scalar, nc.gpsimd]
        # x arrives as [b, (i,k)]; stripe over queues / DMA-capable engines.
        NXS = 8
        XS = IL // NXS
        xsplits = [(q * XS, (q + 1) * XS) for q in range(NXS)]
        for q, (lo, hi) in enumerate(xsplits):
            xs = bass.AP(
                x_kc.tensor, x_kc[:].offset + lo * K,
                [x_kc[:].ap[0], [K, hi - lo], [1, K]])
            eng[q % 3].dma_start(out=xs, in_=x_ext[:, lo:hi, :])
        # [b,(i,k)] -> [b,k,i] with fp16 cast, chunked by i-half so the
        # transposes for it=0 start after only half the x stripes land
        for ih in range(IT):
            lo = ih * 128
            nc.vector.tensor_copy(
                out=x_kc_h[:, :, lo:lo + 128],
                in_=bass.AP(
                    x_kc.tensor, x_kc[:].offset + lo * K,
                    [x_kc[:].ap[0], [1, K], [K, 128]]),
            )
        for chh in range(NCH):
            k, it = divmod(chh, IT)
            p = pst.tile([128, 128], F16, tag="tr")
            nc.tensor.transpose(
                p[:], x_kc_h[:, k, it * 128:(it + 1) * 128], ident_h[:])
            if chh % 2 == 0:
                nc.scalar.copy(out=x_t[:, k, it, :], in_=p[:])
            else:
                nc.vector.tensor_copy(out=x_t[:, k, it, :], in_=p[:])

        w_re = w_ext.rearrange("j (it p) d k -> it p j (d k)", it=IT)
        for it in range(IT):
            for jh in range(4):
                js, je = [(0, 3), (3, 5), (5, 8), (8, 10)][jh]
                eng[(2 * it + jh) % 3].dma_start(
                    out=w_nat[:, it, js:je, :], in_=w_re[it][:, js:je, :],
                )
        nc.vector.tensor_copy(out=w_bf[:], in_=w_nat[:])

        def w_rhs(it: int, j: int, k: int) -> bass.AP:
            """fp16 W slice [(i%128) x d] with d strided over the (d,k) dim."""
            return w_bf[:, it, j, :].rearrange("p (d k) -> p k d", k=K)[:, k, :]

        # ---------- r0: c uniform 1/J, s_raw = sum_i u_hat ----------
        # Runs BEFORE the w_kd relayout so its AllReduce triggers early;
        # x^T is j-independent, so batch all (j,d) into one N=160 stream.
        ps_s = ps_s_pool.tile([128, J, D], F32, tag="sq")
        for chh in range(NCH):
            k, it = divmod(chh, IT)
            rhs_all = w_bf[:, it, :, :].rearrange(
                "p j (d k) -> p k j d", k=K)[:, k, :, :]
            nc.tensor.matmul(
                ps_s[:], lhsT=x_t[:, k, it, :],
                rhs=rhs_all.rearrange("p j d -> p (j d)"),
                start=(chh == 0), stop=(chh == NCH - 1),
            )
        nc.scalar.mul(out=s_stage[:], in_=ps_s[:], mul=1.0 / J)
        nc.sync.dma_start(
            out=ar_in[0][:], in_=s_stage.rearrange("b j d -> b (j d)"))

        # ---------- w_kd relayout (needed only from iteration 1 on) ----------
        # (d,k) -> (k,d) shuffle with fp16 cast (on ACT; DVE is busy)
        nc.scalar.copy(
            out=w_shuf.rearrange("p it j (k d) -> p it j k d", k=K),
            in_=w_nat.rearrange("p it j (d k) -> p it j k d", k=K),
        )
        for it in range(IT):
            for j in range(J):
                p = pst.tile([128, 128], F16, tag="tr")
                nc.tensor.transpose(p[:], w_shuf[:, it, j, :], ident_h[:])
                if j % 2 == 0:
                    nc.scalar.copy(out=w_kd_bf[:, j, it, :], in_=p[:])
                else:
                    nc.vector.tensor_copy(out=w_kd_bf[:, j, it, :], in_=p[:])
        for k in range(K):
            eng[k % 3].dma_start(
                out=w_kd_k[:, k, :, :, :],
                in_=w_kd_bf[k * 16:(k + 1) * 16, :, :, :],
            )

        for r in range(R):
            # ---------- c update + s matmuls ----------
            if r > 0:
                # -- logits increment a_j = sum_k x * (W^T out), in [i, b] --
                for j in range(J):
                    # out^T for this j (tiny fp16 transpose)
                    p = pst.tile([128, 128], F16, tag="tr")
                    nc.tensor.transpose(p[:16, :], out_hb[:, j, :], ident_h[:])
                    nc.scalar.copy(out=outT[:, j, :], in_=p[:16, :])

                    tr1 = rpool.tile([128, KH, IT, 128], F16, tag="tr1")
                    for h in range(2):
                        ps_T = ps_t_pool.tile([128, KH, IT, 128], F32, tag="T")
                        for kk in range(KH):
                            for it in range(IT):
                                nc.tensor.matmul(
                                    ps_T[:, kk, it, :],
                                    lhsT=w_kd_k[:, h * KH + kk, j, it, :],
                                    rhs=outT[:, j, :],
                                    start=True, stop=True,
                                )
                        # evacuate PSUM (cast fp16) on ACT; gpsimd can't read PSUM
                        t_h = tpool.tile([128, KH, IT, 128], F16, tag="t_h")
                        nc.scalar.copy(out=t_h[:], in_=ps_T[:])
                        # p = x * T for this k-half; k-reduce tree 8->4->2->1
                        if h == 0:
                            nc.vector.tensor_tensor(
                                out=tr1[:], in0=x_t[:, 0:KH, :, :], in1=t_h[:],
                                op=OP.mult)
                        else:
                            p_h = tpool.tile([128, KH, IT, 128], F16, tag="p_h")
                            nc.vector.tensor_tensor(
                                out=p_h[:], in0=x_t[:, KH:K, :, :], in1=t_h[:],
                                op=OP.mult)
                            nc.vector.tensor_add(tr1[:], tr1[:], p_h[:])
                    nc.vector.tensor_add(
                        tr1[:, 0:2, :, :], tr1[:, 0:2, :, :], tr1[:, 2:4, :, :])
                    a_t = rpool.tile([128, IT, 128], F16, tag="a")
                    nc.vector.tensor_add(
                        a_t[:], tr1[:, 0, :, :], tr1[:, 1, :, :])
                    # c~_j = (r==1) ? exp(a) : c~_prev * exp(a).  c~ stays
                    # UNnormalized (softmax is scale-invariant per (i,b);
                    # cumulative logits stay within +-5 so fp16 is safe);
                    # 1/Z is folded into x' below instead.
                    if r == 1:
                        nc.scalar.activation(
                            out=c_st[:, :, j, :], in_=a_t[:], func=ACTF.Exp)
                    else:
                        ea_t = rpool.tile([128, IT, 128], F16, tag="ea")
                        nc.scalar.activation(out=ea_t[:], in_=a_t[:], func=ACTF.Exp)
                        nc.vector.tensor_tensor(
                            out=c_st[:, :, j, :], in0=c_st[:, :, j, :],
                            in1=ea_t[:], op=OP.mult)

                # -- softmax over j: z = sum_j c~, x' = x / z --
                nc.vector.tensor_add(zt1[:], c_st[:, :, 0:5, :], c_st[:, :, 5:10, :])
                nc.vector.tensor_add(zt2[:], zt1[:, :, 0:2, :], zt1[:, :, 2:4, :])
                nc.vector.tensor_add(z_t[:], zt2[:, :, 0, :], zt2[:, :, 1, :])
                nc.vector.tensor_add(z_t[:], z_t[:], zt1[:, :, 4, :])
                with nc.allow_low_precision(reason="fp16 1/Z; c has 10-bit mantissa anyway"):
                    nc.vector.reciprocal(out=rz_t[:], in_=z_t[:])
                rzb = bass.AP(
                    rz_t.tensor, rz_t[:].offset,
                    [rz_t.ap[0], [0, K], rz_t.ap[1], rz_t.ap[2]],
                )  # dims [p, k(bcast), it, b]
                nc.vector.tensor_tensor(
                    out=x_p[:], in0=x_t[:], in1=rzb, op=OP.mult)

                # -- per-j: y = c~ * x', s matmuls --
                for q in range((J + 3) // 4):
                    js = list(range(4 * q, min(4 * q + 4, J)))
                    ps_q = ps_s_pool.tile([128, 128], F32, tag="sq")
                    for g, j in enumerate(js):
                        y_j = ypool.tile([128, K, IT, 128], F16, tag="y")
                        cb = bass.AP(
                            c_st.tensor, c_st[:, 0, j, :].offset,
                            [c_st.ap[0], [0, K], c_st.ap[1], c_st.ap[3]],
                        )  # dims [p, k(bcast), it, b]
                        nc.vector.tensor_tensor(
                            out=y_j[:], in0=x_p[:], in1=cb, op=OP.mult)
                        for chh in range(NCH):
                            k, it = divmod(chh, IT)
                            nc.tensor.matmul(
                                ps_q[32 * g:32 * g + 16, :],
                                lhsT=w_rhs(it, j, k), rhs=y_j[:, k, it, :],
                                start=(chh == 0), stop=(chh == NCH - 1),
                                tile_position=(0, 32 * g),
                                skip_group_check=True,
                            )
                    for g, j in enumerate(js):
                        nc.scalar.copy(
                            out=sT_stage[:, j, :], in_=ps_q[32 * g:32 * g + 16, :])
                nc.sync.dma_start(
                    out=ar_in[r].rearrange("b f -> (b f)").rearrange(
                        "(p f) -> p f", p=16),
                    in_=sT_stage.rearrange("d j b -> d (j b)"))

            # ---------- all-reduce s ----------
            if n_cores > 1:
                nc.gpsimd.collective_compute(
                    "AllReduce", OP.add,
                    replica_groups=[list(range(n_cores))],
                    ins=[ar_in[r][:]], outs=[ar_out[r][:]],
                )
                ar_res = ar_out[r]
            else:
                ar_res = ar_in[r]

            if r == 0:
                nc.sync.dma_start(
                    out=s_sb.rearrange("b j d -> b (j d)"), in_=ar_res[:])
            else:
                nc.sync.dma_start(
                    out=sT_sb.rearrange("d j b -> d (j b)"),
                    in_=ar_res.rearrange("b f -> (b f)").rearrange(
                        "(p f) -> p f", p=16))
                for j in range(J):
                    p = pst.tile([128, 128], F16, tag="tr")
                    nc.tensor.transpose(
                        p[:, :16], sT_sb[:, j, :], ident_h[:16, :16])
                    nc.vector.tensor_copy(out=s_sb[:, j, :], in_=p[:, :16])

            # ---------- squash ----------
            v = s_sb  # [128, J, D] fp16
            # s2 = sum_d (v/5)^2 ; mean = sum_d v / D
            nc.vector.scalar_tensor_tensor(
                out=sq_sq[:], in0=v[:], scalar=0.04, in1=v[:],
                op0=OP.mult, op1=OP.mult)
            nc.vector.reduce_sum(out=sq_s2[:], in_=sq_sq[:], axis=AX.X)
            nc.vector.reduce_sum(out=sq_mean[:], in_=v[:], axis=AX.X)
            nc.vector.tensor_scalar_mul(sq_mean[:], sq_mean[:], 1.0 / D)
            # t = v - mean ; var = sum_d t^2 / D
            nc.vector.tensor_sub(sq_t[:], v[:], bcast(sq_mean[:], D))
            nc.vector.tensor_tensor(
                out=sq_sq[:], in0=sq_t[:], in1=sq_t[:], op=OP.mult)
            nc.vector.reduce_sum(out=sq_var[:], in_=sq_sq[:], axis=AX.X)
            nc.vector.tensor_scalar_mul(sq_var[:], sq_var[:], 1.0 / D)
            # rs = 1/sqrt(s2 + K_EPS) = exp(-0.5*ln(s2 + K_EPS))
            nc.scalar.activation(out=sq_ln[:], in_=sq_s2[:], func=ACTF.Ln, bias=eps_k[:])
            nc.scalar.activation(out=sq_rs[:], in_=sq_ln[:], func=ACTF.Exp, scale=-0.5)
            # scale = 0.5*s2/(1+0.5*s2) * rs
            nc.vector.tensor_scalar_mul(sq_u[:], sq_s2[:], 0.5)
            nc.vector.tensor_scalar_add(sq_den[:], sq_u[:], 1.0)
            nc.vector.reciprocal(out=sq_rden[:], in_=sq_den[:])
            nc.vector.tensor_tensor(out=sq_scale[:], in0=sq_u[:], in1=sq_rden[:], op=OP.mult)
            nc.vector.tensor_tensor(out=sq_scale[:], in0=sq_scale[:], in1=sq_rs[:], op=OP.mult)
            # rvar = 1/sqrt(var + NORM_EPS); m2 = scale * rvar; out = t * m2
            nc.scalar.activation(out=sq_ln[:], in_=sq_var[:], func=ACTF.Ln, bias=eps_n[:])
            nc.scalar.activation(out=sq_rs[:], in_=sq_ln[:], func=ACTF.Exp, scale=-0.5)
            nc.vector.tensor_tensor(out=sq_m2[:], in0=sq_scale[:], in1=sq_rs[:], op=OP.mult)
            nc.vector.tensor_tensor(out=out_sb[:], in0=sq_t[:], in1=bcast(sq_m2[:], D), op=OP.mult)

            if r == R - 1:
                nc.sync.dma_start(out=out_ext[:], in_=out_sb[:])
            else:
                # fp16 copy of out for the next iteration's transposes
                nc.scalar.copy(out=out_hb[:], in_=out_sb[:])

    nc.finalize()
    return nc


_cache = {}


def _get_nc(n_cores: int):
    if n_cores not in _cache:
        _cache[n_cores] = build_nc(n_cores)
    return _cache[n_cores]


def kernel(inputs: np.ndarray, W: np.ndarray) -> np.ndarray:
    assert inputs.shape == (B, I, K) and W.shape == (J, I, D, K)
    IL = I // NCORES
    nc = _get_nc(NCORES)
    in_maps = [
        {
            "x": np.ascontiguousarray(inputs[:, c * IL:(c + 1) * IL, :], dtype=np.float32),
            "w": np.ascontiguousarray(W[:, c * IL:(c + 1) * IL, :, :], dtype=np.float32),
        }
        for c in range(NCORES)
    ]
    res = run_bass_kernel_spmd(nc, in_maps, core_ids=list(range(NCORES)))
    return np.asarray(res.results[0]["out"], dtype=np.float32)


# revision 23
# speedup vs baseline: 2.2525x; 2.2525x over previous
"""Trainium2 Bass kernel for nn_CapsuleLayer (dynamic routing capsule layer).

Reference computation:
    u_hat = einsum('jidk,bik->bjid', W, inputs)        # [B,J,I,D]
    b = 0
    for r in 0..2:
        c = softmax_j(b)                               # [B,J,1,I]
        s = einsum('bjoi,bjid->bjod', c, u_hat)        # [B,J,1,D]
        out = squash(s)
        if r < 2: b += einsum('bjod,bjid->bjoi', out, u_hat)
    return out                                         # [B,J,D]

Strategy: shard I (=2048) across 8 cores (I_loc=256), keep full B=128 on
every core.  u_hat (168 MB) is NEVER materialized: both routing
contractions are expressed against W directly:

    s[b,j,d]  = sum_{i,k} (c[b,j,i] * x[b,i,k]) * W[j,i,d,k]   (PE, K=(k,i))
    a[b,j,i]  = sum_k x[b,i,k] * T[b,j,k,i],
    T[b,j,k,i] = sum_d out[b,j,d] * W[j,i,d,k]                 (PE, K=d)

All routing state (c, products, y) lives in the [i%128-partition] layout
so the softmax/y path needs NO transposes; the T matmuls emit [i, b]
tiles directly (stationary = W slices, moving = out^T).  Softmax
normalization is kept as self-normalized state: c <- (c_prev * exp(a)) / Z,
valid because softmax is scale-invariant per (i, b).

One small fp16 AllReduce per routing iteration combines the per-core s
partial sums; a dummy 32-byte AllReduce issued at kernel start absorbs
cross-core launch skew so the real collectives see aligned cores.

Matmul and DVE operands are fp16 (fp32 matmuls cost a 2-way hi/lo split;
2-byte packed DVE ops run in the 2x perf mode).  PE accumulation stays
fp32 in PSUM; squash runs fp32.
"""

import numpy as np
from contextlib import ExitStack

import concourse.bass as bass
import concourse.bacc as bacc
import concourse.tile as tile
from concourse import mybir
from concourse.bass_utils import run_bass_kernel_spmd
from concourse.masks import make_identity

F32 = mybir.dt.float32
F16 = mybir.dt.float16  # 10-bit mantissa, fast PE weight load, DVE 2x mode
AX = mybir.AxisListType
OP = mybir.AluOpType
ACTF = mybir.ActivationFunctionType

B = 128       # batch
I = 2048      # input capsules (sharded)
K = 8         # DIN
J = 10        # output capsules
D = 16        # DOUT
R = 3         # routing iterations
NCORES = 8

K_EPS = 1e-7
NORM_EPS = 1e-6


def bcast(ap: bass.AP, n: int) -> bass.AP:
    """Append a stride-0 (broadcast) innermost free dim of size n."""
    return bass.AP(ap.tensor, ap.offset, [*ap.ap, [0, n]])


def _pin_activation_tables():
    """Make every activation function we use resolve to the one table set
    that contains them all (natural_log_exp_and_others), so the compiler
    emits a single ACT_TABLE_LOAD instead of thrashing between sets."""
    import concourse.hw_specs as hw_specs

    if getattr(bacc, "_capsule_tables_pinned", False):
        return
    orig = hw_specs.get_activation_tables
    mine = {"Exp", "Ln", "Copy", "Identity", "Square"}

    def patched(module_arch):
        tables = dict(orig(module_arch))
        out = {}
        for name, funcs in tables.items():
            if name == "natural_log_exp_and_others":
                out[name] = funcs
            else:
                out[name] = {f for f in funcs if f.name not in mine}
        return out

    bacc.get_activation_tables = patched
    bacc._capsule_tables_pinned = True


def build_nc(n_cores: int = NCORES):
    IL = I // NCORES          # 256 per-core input capsules (also for n_cores=1 sim)
    IT = IL // 128            # 2 partition tiles of i
    NCH = IL * K // 128       # 16 (k,i)-chunks of 128 contraction rows
    KH = K // 2               # T computed in two 4-k halves (PSUM budget)

    _pin_activation_tables()
    nc = bacc.Bacc(num_devices=n_cores)

    x_ext = nc.dram_tensor("x", [B, IL, K], F32, kind="ExternalInput")
    w_ext = nc.dram_tensor("w", [J, IL, D, K], F32, kind="ExternalInput")
    out_ext = nc.dram_tensor("out", [B, J, D], F32, kind="ExternalOutput")
    # fp16 all-reduce payloads, one pair per iteration
    ar_in = [nc.dram_tensor(f"ar_in{r}", [B, J * D], F16) for r in range(R)]
    ar_out = [
        nc.dram_tensor(f"ar_out{r}", [B, J * D], F16, addr_space="Shared")
        for r in range(R)
    ]

    with tile.TileContext(nc) as tc, ExitStack() as ctx:
        sb = ctx.enter_context(tc.tile_pool(name="sb", bufs=1))
        ypool = ctx.enter_context(tc.tile_pool(name="ypool", bufs=3))
        tpool = ctx.enter_context(tc.tile_pool(name="tpool", bufs=3))
        rpool = ctx.enter_context(tc.tile_pool(name="rpool", bufs=3))
        pst = ctx.enter_context(tc.tile_pool(name="pst", bufs=2, space="PSUM"))
        ps_s_pool = ctx.enter_context(tc.tile_pool(name="ps_s", bufs=2, space="PSUM"))
        ps_t_pool = ctx.enter_context(tc.tile_pool(name="ps_t", bufs=2, space="PSUM"))

        # (no dummy collective: the framework's prelude kernel barrier
        # already absorbs cross-core launch skew on the CC engine)
        ident = sb.tile([128, 128], F32)
        make_identity(nc, ident)
        ident_h = sb.tile([128, 128], F16)
        nc.vector.tensor_copy(out=ident_h[:], in_=ident[:])

        # persistent tensors
        x_kc = sb.tile([128, K, IL], F32)            # x[b, k, i] fp32 staging
        x_kc_h = sb.tile([128, K, IL], F16)          # fp16 copy (transpose src)
        x_t = sb.tile([128, K, IT, 128], F16)        # x^T: [(i%128), k, it, b]
        x_p = sb.tile([128, K, IT, 128], F16)        # x' = x / Z (per iteration)
        w_nat = sb.tile([128, IT, J, D * K], F32)    # w[i%128, it, j, (d,k)]
        w_bf = sb.tile([128, IT, J, D * K], F16)     # fp16 copy (s-matmul lhsT)
        w_shuf = sb.tile([128, IT, J, K * D], F16)   # (d,k) -> (k,d) shuffle fp16
        w_kd_k = sb.tile([16, K, J, IT, 128], F16)   # w[d, k, j, it, i] (base-0 rows)
        w_kd_bf = sb.tile([128, J, IT, 128], F16)    # staging for the re-base

        # routing state ([i%128] layout)
        c_st = sb.tile([128, IT, J, 128], F16)       # c[i%128, it, j, b]
        zt1 = sb.tile([128, IT, 5, 128], F16)
        zt2 = sb.tile([128, IT, 2, 128], F16)
        z_t = sb.tile([128, IT, 128], F16)
        rz_t = sb.tile([128, IT, 128], F16)

        # [b] layout state
        s_sb = sb.tile([128, J, D], F16)             # all-reduced s
        s_stage = sb.tile([128, J, D], F16)          # pre-allreduce staging (iter 0)
        sT_stage = sb.tile([16, J, 128], F16)        # pre-allreduce staging (iters 1+)
        sT_sb = sb.tile([16, J, 128], F16)           # all-reduced sT
        out_sb = sb.tile([128, J, D], F32)           # squash output
        out_hb = sb.tile([128, J, D], F16)           # fp16 copy (transpose src)
        outT = sb.tile([16, J, 128], F16)            # out^T: [d, j, b]

        # small squash temps
        sq_s2 = sb.tile([128, J], F32)
        sq_mean = sb.tile([128, J], F32)
        sq_t = sb.tile([128, J, D], F32)
        sq_sq = sb.tile([128, J, D], F32)
        sq_var = sb.tile([128, J], F32)
        sq_ln = sb.tile([128, J], F32)
        sq_rs = sb.tile([128, J], F32)
        sq_u = sb.tile([128, J], F32)
        sq_den = sb.tile([128, J], F32)
        sq_rden = sb.tile([128, J], F32)
        sq_scale = sb.tile([128, J], F32)
        sq_m2 = sb.tile([128, J], F32)
        eps_k = sb.tile([128, 1], F32)
        nc.vector.memset(eps_k[:], K_EPS)
        eps_n = sb.tile([128, 1], F32)
        nc.vector.memset(eps_n[:], NORM_EPS)

        # ---------------- load + relayout ----------------
        eng = [nc.sync, nc.scalar, nc.gpsimd]
        # x arrives as [b, (i,k)]; stripe over queues / DMA-capable engines.
        NXS = 8
        XS = IL // NXS
        xsplits = [(q * XS, (q + 1) * XS) for q in range(NXS)]
        for q, (lo, hi) in enumerate(xsplits):
            xs = bass.AP(
                x_kc.tensor, x_kc[:].offset + lo * K,
                [x_kc[:].ap[0], [K, hi - lo], [1, K]])
            eng[q % 3].dma_start(out=xs, in_=x_ext[:, lo:hi, :])
        # [b,(i,k)] -> [b,k,i] with fp16 cast, chunked by i-half so the
        # transposes for it=0 start after only half the x stripes land
        for ih in range(IT):
            lo = ih * 128
            nc.vector.tensor_copy(
                out=x_kc_h[:, :, lo:lo + 128],
                in_=bass.AP(
                    x_kc.tensor, x_kc[:].offset + lo * K,
                    [x_kc[:].ap[0], [1, K], [K, 128]]),
            )
        for chh in range(NCH):
            k, it = divmod(chh, IT)
            p = pst.tile([128, 128], F16, tag="tr")
            nc.tensor.transpose(
                p[:], x_kc_h[:, k, it * 128:(it + 1) * 128], ident_h[:])
            if chh % 2 == 0:
                nc.scalar.copy(out=x_t[:, k, it, :], in_=p[:])
            else:
                nc.vector.tensor_copy(out=x_t[:, k, it, :], in_=p[:])

        w_re = w_ext.rearrange("j (it p) d k -> it p j (d k)", it=IT)
        for it in range(IT):
            for jh in range(4):
                js, je = [(0, 3), (3, 5), (5, 8), (8, 10)][jh]
                eng[(2 * it + jh) % 3].dma_start(
                    out=w_nat[:, it, js:je, :], in_=w_re[it][:, js:je, :],
                )
        nc.vector.tensor_copy(out=w_bf[:], in_=w_nat[:])

        def w_rhs(it: int, j: int, k: int) -> bass.AP:
            """fp16 W slice [(i%128) x d] with d strided over the (d,k) dim."""
            return w_bf[:, it, j, :].rearrange("p (d k) -> p k d", k=K)[:, k, :]

        # ---------- r0: c uniform 1/J, s_raw = sum_i u_hat ----------
        # Runs BEFORE the w_kd relayout so its AllReduce triggers early;
        # x^T is j-independent, so batch all (j,d) into one N=160 stream.
        ps_s = ps_s_pool.tile([128, J, D], F32, tag="sq")
        for chh in range(NCH):
            k, it = divmod(chh, IT)
            rhs_all = w_bf[:, it, :, :].rearrange(
                "p j (d k) -> p k j d", k=K)[:, k, :, :]
            nc.tensor.matmul(
                ps_s[:], lhsT=x_t[:, k, it, :],
                rhs=rhs_all.rearrange("p j d -> p (j d)"),
                start=(chh == 0), stop=(chh == NCH - 1),
            )
        nc.scalar.mul(out=s_stage[:], in_=ps_s[:], mul=1.0 / J)
        nc.sync.dma_start(
            out=ar_in[0][:], in_=s_stage.rearrange("b j d -> b (j d)"))

        # ---------- w_kd relayout (needed only from iteration 1 on) ----------
        # (d,k) -> (k,d) shuffle with fp16 cast (on ACT; DVE is busy)
        nc.scalar.copy(
            out=w_shuf.rearrange("p it j (k d) -> p it j k d", k=K),
            in_=w_nat.rearrange("p it j (d k) -> p it j k d", k=K),
        )
        for it in range(IT):
            for j in range(J):
                p = pst.tile([128, 128], F16, tag="tr")
                nc.tensor.transpose(p[:], w_shuf[:, it, j, :], ident_h[:])
                if j % 2 == 0:
                    nc.scalar.copy(out=w_kd_bf[:, j, it, :], in_=p[:])
                else:
                    nc.vector.tensor_copy(out=w_kd_bf[:, j, it, :], in_=p[:])
        for k in range(K):
            eng[k % 3].dma_start(
                out=w_kd_k[:, k, :, :, :],
                in_=w_kd_bf[k * 16:(k + 1) * 16, :, :, :],
            )

        for r in range(R):
            # ---------- c update + s matmuls ----------
            if r > 0:
                # -- logits increment a_j = sum_k x * (W^T out), in [i, b] --
                for j in range(J):
                    # out^T for this j (tiny fp16 transpose)
                    p = pst.tile([128, 128], F16, tag="tr")
                    nc.tensor.transpose(p[:16, :], out_hb[:, j, :], ident_h[:])
                    nc.vector.tensor_copy(out=outT[:, j, :], in_=p[:16, :])

                    tr1 = rpool.tile([128, KH, IT, 128], F16, tag="tr1")
                    for h in range(2):
                        ps_T = ps_t_pool.tile([128, KH, IT, 128], F32, tag="T")
                        for kk in range(KH):
                            for it in range(IT):
                                nc.tensor.matmul(
                                    ps_T[:, kk, it, :],
                                    lhsT=w_kd_k[:, h * KH + kk, j, it, :],
                                    rhs=outT[:, j, :],
                                    start=True, stop=True,
                                )
                        # evacuate PSUM (cast fp16) on ACT; gpsimd can't read PSUM
                        t_h = tpool.tile([128, KH, IT, 128], F16, tag="t_h")
                        nc.scalar.copy(out=t_h[:], in_=ps_T[:])
                        # p = x * T for this k-half; k-reduce tree 8->4->2->1
                        if h == 0:
                            nc.vector.tensor_tensor(
                                out=tr1[:], in0=x_t[:, 0:KH, :, :], in1=t_h[:],
                                op=OP.mult)
                        else:
                            p_h = tpool.tile([128, KH, IT, 128], F16, tag="p_h")
                            nc.vector.tensor_tensor(
                                out=p_h[:], in0=x_t[:, KH:K, :, :], in1=t_h[:],
                                op=OP.mult)
                            nc.vector.tensor_add(tr1[:], tr1[:], p_h[:])
                    nc.vector.tensor_add(
                        tr1[:, 0:2, :, :], tr1[:, 0:2, :, :], tr1[:, 2:4, :, :])
                    a_t = rpool.tile([128, IT, 128], F16, tag="a")
                    nc.vector.tensor_add(
                        a_t[:], tr1[:, 0, :, :], tr1[:, 1, :, :])
                    # c~_j = (r==1) ? exp(a) : c~_prev * exp(a).  c~ stays
                    # UNnormalized (softmax is scale-invariant per (i,b);
                    # cumulative logits stay within +-5 so fp16 is safe);
                    # 1/Z is folded into x' below instead.
                    if r == 1:
                        nc.scalar.activation(
                            out=c_st[:, :, j, :], in_=a_t[:], func=ACTF.Exp)
                    else:
                        ea_t = rpool.tile([128, IT, 128], F16, tag="ea")
                        nc.scalar.activation(out=ea_t[:], in_=a_t[:], func=ACTF.Exp)
                        nc.vector.tensor_tensor(
                            out=c_st[:, :, j, :], in0=c_st[:, :, j, :],
                            in1=ea_t[:], op=OP.mult)

                # -- softmax over j: z = sum_j c~, x' = x / z --
                nc.vector.tensor_add(zt1[:], c_st[:, :, 0:5, :], c_st[:, :, 5:10, :])
                nc.vector.tensor_add(zt2[:], zt1[:, :, 0:2, :], zt1[:, :, 2:4, :])
                nc.vector.tensor_add(z_t[:], zt2[:, :, 0, :], zt2[:, :, 1, :])
                nc.vector.tensor_add(z_t[:], z_t[:], zt1[:, :, 4, :])
                with nc.allow_low_precision(reason="fp16 1/Z; c has 10-bit mantissa anyway"):
                    nc.vector.reciprocal(out=rz_t[:], in_=z_t[:])
                rzb = bass.AP(
                    rz_t.tensor, rz_t[:].offset,
                    [rz_t.ap[0], [0, K], rz_t.ap[1], rz_t.ap[2]],
                )  # dims [p, k(bcast), it, b]
                nc.vector.tensor_tensor(
                    out=x_p[:], in0=x_t[:], in1=rzb, op=OP.mult)

                # -- per-j: y = c~ * x', s matmuls --
                for q in range((J + 3) // 4):
                    js = list(range(4 * q, min(4 * q + 4, J)))
                    ps_q = ps_s_pool.tile([128, 128], F32, tag="sq")
                    for g, j in enumerate(js):
                        y_j = ypool.tile([128, K, IT, 128], F16, tag="y")
                        cb = bass.AP(
                            c_st.tensor, c_st[:, 0, j, :].offset,
                            [c_st.ap[0], [0, K], c_st.ap[1], c_st.ap[3]],
                        )  # dims [p, k(bcast), it, b]
                        nc.vector.tensor_tensor(
                            out=y_j[:], in0=x_p[:], in1=cb, op=OP.mult)
                        for chh in range(NCH):
                            k, it = divmod(chh, IT)
                            nc.tensor.matmul(
                                ps_q[32 * g:32 * g + 16, :],
                                lhsT=w_rhs(it, j, k), rhs=y_j[:, k, it, :],
                                start=(chh == 0), stop=(chh == NCH - 1),
                                tile_position=(0, 32 * g),
                                skip_group_check=True,
                            )
                    for g, j in enumerate(js):
                        nc.scalar.copy(
                            out=sT_stage[:, j, :], in_=ps_q[32 * g:32 * g + 16, :])
                nc.sync.dma_start(
                    out=ar_in[r].rearrange("b f -> (b f)").rearrange(
                        "(p f) -> p f", p=16),
                    in_=sT_stage.rearrange("d j b -> d (j b)"))

            # ---------- all-reduce s ----------
            if n_cores > 1:
                nc.gpsimd.collective_compute(
                    "AllReduce", OP.add,
                    replica_groups=[list(range(n_cores))],
                    ins=[ar_in[r][:]], outs=[ar_out[r][:]],
                )
                ar_res = ar_out[r]
            else:
                ar_res = ar_in[r]

            if r == 0:
                nc.sync.dma_start(
                    out=s_sb.rearrange("b j d -> b (j d)"), in_=ar_res[:])
            else:
                nc.sync.dma_start(
                    out=sT_sb.rearrange("d j b -> d (j b)"),
                    in_=ar_res.rearrange("b f -> (b f)").rearrange(
                        "(p f) -> p f", p=16))
                for j in range(J):
                    p = pst.tile([128, 128], F16, tag="tr")
                    nc.tensor.transpose(
                        p[:, :16], sT_sb[:, j, :], ident_h[:16, :16])
                    nc.vector.tensor_copy(out=s_sb[:, j, :], in_=p[:, :16])

            # ---------- squash ----------
            v = s_sb  # [128, J, D] fp16
            # s2 = sum_d (v/5)^2 ; mean = sum_d v / D
            nc.vector.scalar_tensor_tensor(
                out=sq_sq[:], in0=v[:], scalar=0.04, in1=v[:],
                op0=OP.mult, op1=OP.mult)
            nc.vector.reduce_sum(out=sq_s2[:], in_=sq_sq[:], axis=AX.X)
            nc.vector.reduce_sum(out=sq_mean[:], in_=v[:], axis=AX.X)
            nc.vector.tensor_scalar_mul(sq_mean[:], sq_mean[:], 1.0 / D)
            # t = v - mean ; var = sum_d t^2 / D
            nc.vector.tensor_sub(sq_t[:], v[:], bcast(sq_mean[:], D))
            nc.vector.tensor_tensor(
                out=sq_sq[:], in0=sq_t[:], in1=sq_t[:], op=OP.mult)
            nc.vector.reduce_sum(out=sq_var[:], in_=sq_sq[:], axis=AX.X)
            nc.vector.tensor_scalar_mul(sq_var[:], sq_var[:], 1.0 / D)
            # rs = 1/sqrt(s2 + K_EPS) = exp(-0.5*ln(s2 + K_EPS))
            nc.scalar.activation(out=sq_ln[:], in_=sq_s2[:], func=ACTF.Ln, bias=eps_k[:])
            nc.scalar.activation(out=sq_rs[:], in_=sq_ln[:], func=ACTF.Exp, scale=-0.5)
            # scale = 0.5*s2/(1+0.5*s2) * rs
            nc.vector.tensor_scalar_mul(sq_u[:], sq_s2[:], 0.5)
            nc.vector.tensor_scalar_add(sq_den[:], sq_u[:], 1.0)
            nc.vector.reciprocal(out=sq_rden[:], in_=sq_den[:])
            nc.vector.tensor_tensor(out=sq_scale[:], in0=sq_u[:], in1=sq_rden[:], op=OP.mult)
            nc.vector.tensor_tensor(out=sq_scale[:], in0=sq_scale[:], in1=sq_rs[:], op=OP.mult)
            # rvar = 1/sqrt(var + NORM_EPS); m2 = scale * rvar; out = t * m2
            nc.scalar.activation(out=sq_ln[:], in_=sq_var[:], func=ACTF.Ln, bias=eps_n[:])
            nc.scalar.activation(out=sq_rs[:], in_=sq_ln[:], func=ACTF.Exp, scale=-0.5)
            nc.vector.tensor_tensor(out=sq_m2[:], in0=sq_scale[:], in1=sq_rs[:], op=OP.mult)
            nc.vector.tensor_tensor(out=out_sb[:], in0=sq_t[:], in1=bcast(sq_m2[:], D), op=OP.mult)

            if r == R - 1:
                nc.sync.dma_start(out=out_ext[:], in_=out_sb[:])
            else:
                # fp16 copy of out for the next iteration's transposes
                nc.scalar.copy(out=out_hb[:], in_=out_sb[:])

    nc.finalize()
    return nc


_cache = {}


def _get_nc(n_cores: int):
    if n_cores not in _cache:
        _cache[n_cores] = build_nc(n_cores)
    return _cache[n_cores]


def kernel(inputs: np.ndarray, W: np.ndarray) -> np.ndarray:
    assert inputs.shape == (B, I, K) and W.shape == (J, I, D, K)
    IL = I // NCORES
    nc = _get_nc(NCORES)
    in_maps = [
        {
            "x": np.ascontiguousarray(inputs[:, c * IL:(c + 1) * IL, :], dtype=np.float32),
            "w": np.ascontiguousarray(W[:, c * IL:(c + 1) * IL, :, :], dtype=np.float32),
        }
        for c in range(NCORES)
    ]
    res = run_bass_kernel_spmd(nc, in_maps, core_ids=list(range(NCORES)))
    return np.asarray(res.results[0]["out"], dtype=np.float32)
